# revision 1
# baseline (speedup 1.0000x reference)
"""MoELoRALinear Trainium2 kernel (8-core data-parallel, Bass/Tile).

Math (per token t, out feature o):
    out[t,o] = x[t,:] @ base_w[o,:] + base_b[o]
             + sum_e softmax_e(x[t,:] @ router_w[e,:]) * SCALE
               * sum_r (x[t,:] @ A[e,r,:]) * B[e,o,r]

Strategy:
  - 8192 tokens sharded 8 ways (1024 tokens/core); weights replicated.
  - Host-side layout prep only (transposes/reshapes so every device DMA is
    per-partition contiguous); all FLOPs run on device.
  - Matmuls in float32r (fp32 bits, fast PE mode), fp32 PSUM accumulate.
  - Per 128-token chunk: Y = x @ [A;router].T (N=36 matmul group), softmax
    over 4 router cols on DVE/ACT, gates*SCALE applied to the rank-32
    projection, PE-transposed to [32,128]; the gated projection plus a
    ones-row (for bias) is one extra K=33 matmul accumulated into the same
    PSUM tile as the 16 base-matmul K-chunks.
"""

import os
import sys

import numpy as np

import concourse.bacc as bacc
import concourse.bass as bass
import concourse.mybir as mybir
from concourse import masks
from concourse.bass_utils import run_bass_kernel_spmd
from concourse.tile import TileContext

SCALE = 16.0 / 8.0  # alpha / r

N_CORES = 8
TOK = 8192  # 4 * 2048 tokens total
TPC = TOK // N_CORES  # tokens per core = 1024
D = 2048  # in features
O = 2048  # out features
E = 4
R = 8
ER = E * R  # 32
J = ER + E  # 36: rank-proj cols + router cols
DC = D // 128  # 16 contraction chunks
OCW = 512  # out-feature chunk width (one PSUM bank)
OC = O // OCW  # 4
TC = TPC // 128  # 8 token chunks per core

F32 = mybir.dt.float32
F32R = mybir.dt.float32r
BF16 = mybir.dt.bfloat16

# Results of the last device run (for test harness inspection).
last_run_info: dict = {}

_cached = None


def _build_program():
    nc = bacc.Bacc()

    xt_d = nc.declare_dram_parameter("xt", [128, DC * TPC], F32R, isOutput=False)
    wt_d = nc.declare_dram_parameter("wt", [OC, 128, DC * OCW], F32R, isOutput=False)
    w1t_d = nc.declare_dram_parameter("w1t", [128, DC * J], F32R, isOutput=False)
    bcat_d = nc.declare_dram_parameter("bcat", [ER + 1, O], BF16, isOutput=False)
    out_d = nc.declare_dram_parameter("out", [OC, TC, 128, OCW], F32, isOutput=True)

    with TileContext(nc) as tc:
        with (
            tc.tile_pool(name="cpool", bufs=1) as cpool,
            tc.tile_pool(name="wpool", bufs=2) as wpool,
            tc.tile_pool(name="spool", bufs=3) as spool,
            tc.tile_pool(name="opool", bufs=6) as opool,
            tc.tile_pool(name="mpsum", bufs=6, space="PSUM") as mpsum,
            tc.tile_pool(name="ypsum", bufs=2, space="PSUM") as ypsum,
        ):
            # Small tables go on the ACT HWDGE ring so the x-chunk stream on
            # the sync ring starts immediately (each small DMA costs ~2us of
            # fixed latency; serializing them ahead of x wastes ~10us).
            w1tr = cpool.tile([128, DC * J], F32R)
            nc.scalar.dma_start(out=w1tr, in_=w1t_d[:, :])
            bcatr = cpool.tile([ER + 1, O], BF16)
            nc.scalar.dma_start(out=bcatr, in_=bcat_d[:, :])
            xtr = cpool.tile([128, DC * TPC], F32R)

            def load_x(dc):
                nc.sync.dma_start(
                    out=xtr[:, dc * TPC : (dc + 1) * TPC],
                    in_=xt_d[:, dc * TPC : (dc + 1) * TPC],
                )

            def load_w(wtile, oc):
                for k in range(4):
                    nc.sync.dma_start(
                        out=wtile[:, k * 4 * OCW : (k + 1) * 4 * OCW],
                        in_=wt_d[oc, :, k * 4 * OCW : (k + 1) * 4 * OCW],
                    )

            # Interleave: 4 x-chunks (2MB), then the matching 1MB wt[0]
            # piece, so base matmuls for the first PSUM-bank wave unlock
            # every ~3MB of arrival instead of after 12MB.
            wts = {0: wpool.tile([128, DC * OCW], F32R, name="wtr0", tag="wtr")}
            for k in range(4):
                for dc in range(4 * k, 4 * k + 4):
                    load_x(dc)
                nc.sync.dma_start(
                    out=wts[0][:, k * 4 * OCW : (k + 1) * 4 * OCW],
                    in_=wt_d[0, :, k * 4 * OCW : (k + 1) * 4 * OCW],
                )
            wts[1] = wpool.tile([128, DC * OCW], F32R, name="wtr1", tag="wtr")
            load_w(wts[1], 1)

            ident = cpool.tile([128, 128], F32)
            masks.make_identity(nc, ident)
            # Gated projection, transposed, + ones row (bias), all 8 t-chunks.
            # bf16: the LoRA term is ~6% of the output, so bf16 noise here is
            # negligible next to the fp32r base matmul.
            vwtr = cpool.tile([ER + 1, TC * 128], BF16)
            nc.vector.memset(vwtr[ER : ER + 1, :], 1.0)

            # --- Router/LoRA-down phase, transposed: YT[j, t] with
            # YT[0:32]=proj, YT[32:36]=logits. 32 wide-N matmuls instead of
            # 128 narrow ones (fp32r matmul cost is LDWEIGHTS-bound, ~equal
            # for N=36 and N=512).
            ytps = [
                ypsum.tile([J, 512], F32, name=f"ytps{th}", tag="yb")
                for th in range(2)
            ]
            # Wave A: base-matmul groups for oc=0, t=0..5 (6 PSUM banks).
            # Emit YT + wave-A matmuls interleaved in 4-chunk blocks matching
            # the DMA arrival order (x block k, then wt[0] piece k), so the
            # PE's in-order stream ladders along the arriving data.
            psA = {
                t: mpsum.tile([128, OCW], F32, name=f"ps0_{t}", tag="ps")
                for t in range(6)
            }
            for k in range(4):
                for dc in range(4 * k, 4 * k + 4):
                    for th in range(2):
                        nc.tensor.matmul(
                            ytps[th],
                            lhsT=w1tr[:, dc * J : (dc + 1) * J],
                            rhs=xtr[:, dc * TPC + th * 512 : dc * TPC + (th + 1) * 512],
                            start=(dc == 0),
                            stop=(dc == DC - 1),
                        )
                for dc in range(4 * k, 4 * k + 4):
                    for t in range(6):
                        nc.tensor.matmul(
                            psA[t],
                            lhsT=xtr[:, dc * TPC + t * 128 : dc * TPC + (t + 1) * 128],
                            rhs=wts[0][:, dc * OCW : (dc + 1) * OCW],
                            start=(dc == 0),
                            stop=False,
                        )
            yt_sb = cpool.tile([J, TPC], F32)
            for th in range(2):
                nc.vector.tensor_copy(yt_sb[:, th * 512 : (th + 1) * 512], ytps[th])

            # Per 128-token chunk: transpose YT slice to [t, j], softmax the
            # 4 router columns, gate-and-scale the 32 projection columns,
            # transpose back, append as bf16 into the fused-accum lhsT.
            for t in range(TC):
                yps = ypsum.tile([128, J], F32, name=f"tps{t}", tag="yb")
                nc.tensor.transpose(
                    yps, yt_sb[:, t * 128 : (t + 1) * 128], ident[0:J, 0:J]
                )
                nmax = spool.tile([128, 1], F32, tag="nmax")
                nc.vector.reduce_max(
                    nmax, yps[:, ER:J], axis=mybir.AxisListType.X, negate=True
                )
                e4 = spool.tile([128, E], F32, tag="e4")
                nc.scalar.activation(
                    e4,
                    yps[:, ER:J],
                    mybir.ActivationFunctionType.Exp,
                    bias=nmax[:, 0:1],
                    scale=1.0,
                )
                ssum = spool.tile([128, 1], F32, tag="ssum")
                nc.vector.reduce_sum(ssum, e4, axis=mybir.AxisListType.X)
                rinv = spool.tile([128, 1], F32, tag="rinv")
                nc.vector.reciprocal(rinv, ssum)
                ge = spool.tile([128, E], F32, tag="ge")
                nc.vector.tensor_scalar(
                    out=ge,
                    in0=e4,
                    scalar1=rinv[:, 0:1],
                    scalar2=SCALE,
                    op0=mybir.AluOpType.mult,
                    op1=mybir.AluOpType.mult,
                )
                vw = spool.tile([128, ER], F32, tag="vw")
                for e in range(E):
                    nc.vector.tensor_scalar_mul(
                        vw[:, e * R : (e + 1) * R],
                        yps[:, e * R : (e + 1) * R],
                        ge[:, e : e + 1],
                    )
                tps = ypsum.tile([ER, 128], F32, name=f"vtps{t}", tag="yb")
                nc.tensor.transpose(tps, vw, ident)
                nc.vector.tensor_copy(vwtr[0:ER, t * 128 : (t + 1) * 128], tps)

            # --- Main phase: base matmul + fused LoRA-up/bias accumulation
            def close_group(ps, oc, t):
                nc.tensor.matmul(
                    ps,
                    lhsT=vwtr[:, t * 128 : (t + 1) * 128],
                    rhs=bcatr[:, oc * OCW : (oc + 1) * OCW],
                    start=False,
                    stop=True,
                )
                ot = opool.tile([128, OCW], F32, tag="ot")
                nc.vector.tensor_copy(ot, ps)
                nc.sync.dma_start(out=out_d[oc, t], in_=ot)

            def full_group(wtr, oc, t):
                ps = mpsum.tile([128, OCW], F32, name=f"ps{oc}_{t}", tag="ps")
                for dc in range(DC):
                    nc.tensor.matmul(
                        ps,
                        lhsT=xtr[:, dc * TPC + t * 128 : dc * TPC + (t + 1) * 128],
                        rhs=wtr[:, dc * OCW : (dc + 1) * OCW],
                        start=(dc == 0),
                        stop=False,
                    )
                close_group(ps, oc, t)

            # close wave A, then the two remaining oc=0 groups
            for t in range(6):
                close_group(psA[t], 0, t)
            for t in range(6, TC):
                full_group(wts[0], 0, t)
            for oc in range(1, OC):
                if oc >= 2:
                    wts[oc] = wpool.tile(
                        [128, DC * OCW], F32R, name=f"wtr{oc}", tag="wtr"
                    )
                    load_w(wts[oc], oc)
                for t in range(TC):
                    full_group(wts[oc], oc, t)

    nc.compile()
    return nc


def _round_fp32r(a):
    """Round fp32 array to fp32r (RNE to 11 mantissa bits), matching
    walrus's fp32_to_fp32r. The PE's fast fp32r matmul mode requires
    operands pre-rounded to this grid."""
    b = np.ascontiguousarray(a, dtype=np.float32).view(np.uint32)
    lsb = (b >> np.uint32(12)) & np.uint32(1)
    r = (b + np.uint32(0x7FF) + lsb) & np.uint32(0xFFFFF000)
    return r.view(np.float32)


def _prep_inputs(x, base_w, base_b, A, B, router_w):
    """Host-side layout prep: build per-partition-contiguous DMA images."""
    x2 = np.ascontiguousarray(x, dtype=np.float32).reshape(TOK, D)
    # xt[core][p, dc*TPC + t] = x2[core*TPC + t, dc*128 + p]
    xv = x2.reshape(N_CORES, TPC, DC, 128)
    xt = np.ascontiguousarray(xv.transpose(0, 3, 2, 1)).reshape(N_CORES, 128, DC * TPC)

    # wt[oc, p, dc*OCW + o] = base_w[oc*OCW + o, dc*128 + p]
    wv = np.ascontiguousarray(base_w, dtype=np.float32).reshape(OC, OCW, DC, 128)
    wt = np.ascontiguousarray(wv.transpose(0, 3, 2, 1)).reshape(OC, 128, DC * OCW)

    # W1 = [A flattened to 32 rows; router_w 4 rows] over D
    W1 = np.concatenate(
        [np.asarray(A, dtype=np.float32).reshape(ER, D), np.asarray(router_w, np.float32)],
        axis=0,
    )  # [36, D]
    w1v = W1.reshape(J, DC, 128)
    w1t = np.ascontiguousarray(w1v.transpose(2, 1, 0)).reshape(128, DC * J)

    # bcat rows 0..31: B[e, o, r] -> [er, o]; row 32: base_b  (bf16)
    import ml_dtypes

    bc = np.concatenate(
        [
            np.asarray(B, dtype=np.float32).transpose(0, 2, 1).reshape(ER, O),
            np.asarray(base_b, dtype=np.float32)[None, :],
        ],
        axis=0,
    ).astype(ml_dtypes.bfloat16)  # [33, O]
    return _round_fp32r(xt), _round_fp32r(wt), _round_fp32r(w1t), bc


def kernel(x, base_w, base_b, A, B, router_w):
    global _cached
    if _cached is None:
        _cached = _build_program()
    nc = _cached

    xt, wt, w1t, bc = _prep_inputs(x, base_w, base_b, A, B, router_w)

    in_maps = [
        {"xt": xt[c], "wt": wt, "w1t": w1t, "bcat": bc} for c in range(N_CORES)
    ]
    core_ids = list(range(N_CORES))

    profile = os.environ.get("KERNEL_PROFILE", "0") == "1"
    res = run_bass_kernel_spmd(nc, in_maps, core_ids, trace=profile)

    last_run_info.clear()
    last_run_info["exec_time_ns"] = res.exec_time_ns
    last_run_info["mean_exec_time_ns"] = res.mean_exec_time_ns
    last_run_info["instructions_and_trace"] = res.instructions_and_trace
    last_run_info["profile_json"] = res.profile_json

    # out[core] shape [OC, TC, 128, OCW] -> tokens x features
    full = np.empty((TOK, O), dtype=np.float32)
    for c in range(N_CORES):
        buf = res.results[c]["out"]  # [OC, TC, 128, OCW]
        full[c * TPC : (c + 1) * TPC] = (
            buf.transpose(1, 2, 0, 3).reshape(TPC, O)
        )
    return full.reshape(4, 2048, 2048)



# revision 3
# speedup vs baseline: 1.0078x; 1.0078x over previous
"""MoELoRALinear Trainium2 kernel (8-core data-parallel, Bass/Tile).

Math (per token t, out feature o):
    out[t,o] = x[t,:] @ base_w[o,:] + base_b[o]
             + sum_e softmax_e(x[t,:] @ router_w[e,:]) * SCALE
               * sum_r (x[t,:] @ A[e,r,:]) * B[e,o,r]

Strategy:
  - 8192 tokens sharded 8 ways (1024 tokens/core); weights replicated.
  - All operands shipped as bf16 (half the DMA bytes of fp32r; same PE
    throughput: 1 col/cycle). Output returned bf16, upcast on host.
    Norm rel-err ~2e-3, well inside the 2e-2 gate.
  - Per 128-col chunk: YT[j, tok] = [A;router].T @ x accumulated over all
    16 K-chunks (wide-N matmuls), interleaved with a 4-token-chunk base
    matmul wave (4+2 PSUM banks).
  - Softmax/gating WITHOUT PE transposes: exp on ACT straight from PSUM
    (no max-sub needed: logits are ~N(0,1)); per-token sums / broadcast
    done with tiny matmuls (ones / expansion-matrix lhsT); division and
    gating on DVE. The gated rank-32 projection lands directly in the
    [33, tok] lhsT layout the close matmuls need.
  - Close matmul (K=33: gated proj + ones row for bias) accumulates into
    the same PSUM group as the 16 base K-chunks.
  - DMA rings split: scalar=weights/tables, gpsimd=x stream, sync=outputs.
  - 4 warmup matmuls on a zero tile ramp the PE DVFS p-state during the
    fixed ~7us NEFF prologue + first x-chunk DMA.
"""

import os

import numpy as np

import concourse.bacc as bacc
import concourse.bass as bass
import concourse.mybir as mybir
from concourse.bass_utils import run_bass_kernel_spmd
from concourse.tile import TileContext

SCALE = 16.0 / 8.0  # alpha / r

N_CORES = 8
TOK = 8192  # 4 * 2048 tokens total
TPC = TOK // N_CORES  # tokens per core = 1024
D = 2048  # in features
O = 2048  # out features
E = 4
R = 8
ER = E * R  # 32
J = ER + E  # 36: rank-proj rows + router rows
DC = D // 128  # 16 contraction chunks
OCW = 512  # out-feature chunk width (one PSUM bank)
OC = O // OCW  # 4
TC = TPC // 128  # 8 token chunks per core
WAVE = 4  # wave-A token chunks (PSUM banks: WAVE + 2 spare + 2 ypsum)

F32 = mybir.dt.float32
BF16 = mybir.dt.bfloat16

# Results of the last device run (for test harness inspection).
last_run_info: dict = {}

_cached = None


def _build_program():
    nc = bacc.Bacc()

    xt_d = nc.declare_dram_parameter("xt", [128, DC * TPC], BF16, isOutput=False)
    wt_d = nc.declare_dram_parameter("wt", [OC, 128, DC * OCW], BF16, isOutput=False)
    w1t_d = nc.declare_dram_parameter("w1t", [128, DC * J], BF16, isOutput=False)
    bcat_d = nc.declare_dram_parameter("bcat", [ER + 1, O], BF16, isOutput=False)
    cst_d = nc.declare_dram_parameter("cst", [E, ER + 1], BF16, isOutput=False)
    out_d = nc.declare_dram_parameter("out", [OC, TC, 128, OCW], BF16, isOutput=True)

    MUL = mybir.AluOpType.mult

    with TileContext(nc) as tc:
        with (
            tc.tile_pool(name="cpool", bufs=1) as cpool,
            tc.tile_pool(name="wpool", bufs=4) as wpool,
            tc.tile_pool(name="opool", bufs=6) as opool,
            tc.tile_pool(name="mpsum", bufs=6, space="PSUM") as mpsum,
            tc.tile_pool(name="ypsum", bufs=2, space="PSUM") as ypsum,
        ):
            # Small tables first on the scalar (ACT) ring, then the base
            # weights; x streams on the gpsimd ring; outputs on sync.
            cstr = cpool.tile([E, ER + 1], BF16)
            nc.scalar.dma_start(out=cstr, in_=cst_d[:, :])
            w1tr = cpool.tile([128, DC * J], BF16)
            nc.scalar.dma_start(out=w1tr, in_=w1t_d[:, :])
            bcatr = cpool.tile([ER + 1, O], BF16)
            nc.scalar.dma_start(out=bcatr, in_=bcat_d[:, :])

            wts = {
                oc: wpool.tile([128, DC * OCW], BF16, name=f"wtr{oc}", tag="wtr")
                for oc in range(OC)
            }
            # w0 in 4 k-block pieces (wave A unlocks per piece), w1..3 in
            # halves behind it.
            for k in range(4):
                nc.scalar.dma_start(
                    out=wts[0][:, k * 4 * OCW : (k + 1) * 4 * OCW],
                    in_=wt_d[0, :, k * 4 * OCW : (k + 1) * 4 * OCW],
                )
            for oc in range(1, OC):
                for h in range(2):
                    nc.scalar.dma_start(
                        out=wts[oc][:, h * 8 * OCW : (h + 1) * 8 * OCW],
                        in_=wt_d[oc, :, h * 8 * OCW : (h + 1) * 8 * OCW],
                    )

            xtr = cpool.tile([128, DC * TPC], BF16)

            def load_x(d0, nd):
                nc.gpsimd.dma_start(
                    out=xtr[:, d0 * TPC : (d0 + nd) * TPC],
                    in_=xt_d[:, d0 * TPC : (d0 + nd) * TPC],
                )

            for dc in range(4):
                load_x(dc, 1)
            for dp in range(2, 8):
                load_x(2 * dp, 2)

            # Warmup: ramp the PE p-state on a zero tile while the first
            # x chunk is still in flight.
            warm_sb = cpool.tile([128, OCW], BF16)
            nc.vector.memset(warm_sb, 0.0)
            vwtr = cpool.tile([ER + 1, TPC], BF16)
            nc.vector.memset(vwtr[ER : ER + 1, :], 1.0)
            warmps = ypsum.tile([128, OCW], F32, name="warmps", tag="yb")
            for _ in range(4):
                nc.tensor.matmul(
                    warmps, lhsT=warm_sb[:, 0:128], rhs=warm_sb, start=True, stop=True
                )

            # --- Router/LoRA-down phase: YT[j, tok] accumulated over all
            # dc, interleaved with wave-A base matmuls in DMA arrival order.
            ytps = [
                ypsum.tile([J, 512], F32, name=f"ytps{th}", tag="yb")
                for th in range(2)
            ]
            psA = {
                t: mpsum.tile([128, OCW], F32, name=f"ps0_{t}", tag="ps")
                for t in range(WAVE)
            }
            for k in range(4):
                for dc in range(4 * k, 4 * k + 4):
                    for th in range(2):
                        nc.tensor.matmul(
                            ytps[th],
                            lhsT=w1tr[:, dc * J : (dc + 1) * J],
                            rhs=xtr[:, dc * TPC + th * 512 : dc * TPC + (th + 1) * 512],
                            start=(dc == 0),
                            stop=(dc == DC - 1),
                        )
                for dc in range(4 * k, 4 * k + 4):
                    for t in range(WAVE):
                        nc.tensor.matmul(
                            psA[t],
                            lhsT=xtr[:, dc * TPC + t * 128 : dc * TPC + (t + 1) * 128],
                            rhs=wts[0][:, dc * OCW : (dc + 1) * OCW],
                            start=(dc == 0),
                            stop=False,
                        )

            # --- Gating chain (no PE transposes).
            # u = exp(logits) straight off PSUM; s/SCALE via ones-matmul;
            # r = SCALE/s via DVE recip; g = u*r broadcast to [4,tok] via
            # ones-matmul; gb = E8.T@g broadcasts per-expert gates to the
            # 32 rank rows; vw = Y * gb lands directly in [32, tok] bf16.
            y_sb = cpool.tile([ER, TPC], BF16)
            u_sb = cpool.tile([E, TPC], BF16)
            r_sb = cpool.tile([1, TPC], BF16)
            g_sb = cpool.tile([E, TPC], BF16)
            for th in range(2):
                nc.vector.tensor_copy(
                    y_sb[:, th * 512 : (th + 1) * 512], ytps[th][0:ER, :]
                )
                nc.scalar.activation(
                    u_sb[:, th * 512 : (th + 1) * 512],
                    ytps[th][ER:J, :],
                    mybir.ActivationFunctionType.Exp,
                )

            def open_group(wtr, t, name):
                ps = mpsum.tile([128, OCW], F32, name=name, tag="ps")
                for dc in range(DC):
                    nc.tensor.matmul(
                        ps,
                        lhsT=xtr[:, dc * TPC + t * 128 : dc * TPC + (t + 1) * 128],
                        rhs=wtr[:, dc * OCW : (dc + 1) * OCW],
                        start=(dc == 0),
                        stop=False,
                    )
                return ps

            def open_group_part(ps, wtr, t, dcs):
                for dc in dcs:
                    nc.tensor.matmul(
                        ps,
                        lhsT=xtr[:, dc * TPC + t * 128 : dc * TPC + (t + 1) * 128],
                        rhs=wtr[:, dc * OCW : (dc + 1) * OCW],
                        start=(dc == 0),
                        stop=False,
                    )

            # t4 group fills the PE while exp/sums land.
            psA[WAVE] = open_group(wts[0], WAVE, f"ps0_{WAVE}")

            sums = []
            for th in range(2):
                s = ypsum.tile([1, 512], F32, name=f"sums{th}", tag="yb")
                nc.tensor.matmul(
                    s,
                    lhsT=cstr[0:E, ER : ER + 1],
                    rhs=u_sb[:, th * 512 : (th + 1) * 512],
                    start=True,
                    stop=True,
                )
                sums.append(s)
            with nc.allow_low_precision(reason="gates only need ~1% accuracy"):
                for th in range(2):
                    nc.vector.reciprocal(
                        r_sb[:, th * 512 : (th + 1) * 512], sums[th]
                    )

            # t5 group split around the rb4/gb matmuls to hide DVE latency.
            psA[WAVE + 1] = mpsum.tile([128, OCW], F32, name=f"ps0_{WAVE+1}", tag="ps")
            open_group_part(psA[WAVE + 1], wts[0], WAVE + 1, range(0, 8))

            rb4 = []
            for th in range(2):
                rb = ypsum.tile([E, 512], F32, name=f"rb4{th}", tag="yb")
                nc.tensor.matmul(
                    rb,
                    lhsT=cstr[0:1, 0:E],
                    rhs=r_sb[:, th * 512 : (th + 1) * 512],
                    start=True,
                    stop=True,
                )
                rb4.append(rb)
            for th in range(2):
                nc.vector.tensor_tensor(
                    g_sb[:, th * 512 : (th + 1) * 512],
                    u_sb[:, th * 512 : (th + 1) * 512],
                    rb4[th],
                    op=MUL,
                )

            open_group_part(psA[WAVE + 1], wts[0], WAVE + 1, range(8, DC))

            gbs = []
            for th in range(2):
                gb = ypsum.tile([ER, 512], F32, name=f"gb{th}", tag="yb")
                nc.tensor.matmul(
                    gb,
                    lhsT=cstr[0:E, 0:ER],
                    rhs=g_sb[:, th * 512 : (th + 1) * 512],
                    start=True,
                    stop=True,
                )
                gbs.append(gb)
            for th in range(2):
                nc.vector.tensor_tensor(
                    vwtr[0:ER, th * 512 : (th + 1) * 512],
                    y_sb[:, th * 512 : (th + 1) * 512],
                    gbs[th],
                    op=MUL,
                )

            # --- Close: fused LoRA-up + bias matmul, cast to bf16, DMA out.
            def close_group(ps, oc, t):
                nc.tensor.matmul(
                    ps,
                    lhsT=vwtr[:, t * 128 : (t + 1) * 128],
                    rhs=bcatr[:, oc * OCW : (oc + 1) * OCW],
                    start=False,
                    stop=True,
                )
                ot = opool.tile([128, OCW], BF16, tag="ot")
                nc.vector.tensor_copy(ot, ps)
                nc.sync.dma_start(out=out_d[oc, t], in_=ot)

            for t in range(WAVE + 2):
                close_group(psA[t], 0, t)
            for t in range(WAVE + 2, TC):
                close_group(open_group(wts[0], t, f"ps0_{t}"), 0, t)
            for oc in range(1, OC):
                for t in range(TC):
                    close_group(open_group(wts[oc], t, f"ps{oc}_{t}"), oc, t)

    nc.compile()
    return nc


def _prep_inputs(x, base_w, base_b, A, B, router_w):
    """Host-side layout prep: per-partition-contiguous bf16 DMA images."""
    import ml_dtypes

    bf16 = ml_dtypes.bfloat16

    x2 = np.ascontiguousarray(x, dtype=np.float32).reshape(TOK, D)
    # xt[core][p, dc*TPC + t] = x2[core*TPC + t, dc*128 + p]
    xv = x2.reshape(N_CORES, TPC, DC, 128)
    xt = (
        np.ascontiguousarray(xv.transpose(0, 3, 2, 1))
        .reshape(N_CORES, 128, DC * TPC)
        .astype(bf16)
    )

    # wt[oc, p, dc*OCW + o] = base_w[oc*OCW + o, dc*128 + p]
    wv = np.ascontiguousarray(base_w, dtype=np.float32).reshape(OC, OCW, DC, 128)
    wt = (
        np.ascontiguousarray(wv.transpose(0, 3, 2, 1))
        .reshape(OC, 128, DC * OCW)
        .astype(bf16)
    )

    # W1 = [A flattened to 32 rows; router_w 4 rows] over D
    W1 = np.concatenate(
        [
            np.asarray(A, dtype=np.float32).reshape(ER, D),
            np.asarray(router_w, np.float32),
        ],
        axis=0,
    )  # [36, D]
    w1t = (
        np.ascontiguousarray(W1.reshape(J, DC, 128).transpose(2, 1, 0))
        .reshape(128, DC * J)
        .astype(bf16)
    )

    # bcat rows 0..31: B[e, o, r] -> [er, o]; row 32: base_b
    bc = np.concatenate(
        [
            np.asarray(B, dtype=np.float32).transpose(0, 2, 1).reshape(ER, O),
            np.asarray(base_b, dtype=np.float32)[None, :],
        ],
        axis=0,
    ).astype(bf16)  # [33, O]

    # cst[:, :32] = per-expert expansion (E8), cst[:, 32] = 1/SCALE so the
    # ones-matmul computes s/SCALE and recip gives SCALE/s directly.
    cst = np.zeros((E, ER + 1), np.float32)
    for e in range(E):
        cst[e, e * R : (e + 1) * R] = 1.0
    cst[:, ER] = 1.0 / SCALE
    cst = cst.astype(bf16)

    return xt, wt, w1t, bc, cst


def kernel(x, base_w, base_b, A, B, router_w):
    global _cached
    if _cached is None:
        _cached = _build_program()
    nc = _cached

    xt, wt, w1t, bc, cst = _prep_inputs(x, base_w, base_b, A, B, router_w)

    in_maps = [
        {"xt": xt[c], "wt": wt, "w1t": w1t, "bcat": bc, "cst": cst}
        for c in range(N_CORES)
    ]
    core_ids = list(range(N_CORES))

    profile = os.environ.get("KERNEL_PROFILE", "0") == "1"
    res = run_bass_kernel_spmd(nc, in_maps, core_ids, trace=profile)

    last_run_info.clear()
    last_run_info["exec_time_ns"] = res.exec_time_ns
    last_run_info["mean_exec_time_ns"] = res.mean_exec_time_ns
    last_run_info["instructions_and_trace"] = res.instructions_and_trace
    last_run_info["profile_json"] = res.profile_json

    # out[core] shape [OC, TC, 128, OCW] bf16 -> tokens x features fp32
    full = np.empty((TOK, O), dtype=np.float32)
    for c in range(N_CORES):
        buf = res.results[c]["out"].astype(np.float32)  # [OC, TC, 128, OCW]
        full[c * TPC : (c + 1) * TPC] = buf.transpose(1, 2, 0, 3).reshape(TPC, O)
    return full.reshape(4, 2048, 2048)


# revision 13
# speedup vs baseline: 1.0950x; 1.0865x over previous
"""MoELoRALinear Trainium2 kernel (8-core data-parallel, Bass/Tile).

Math (per token t, out feature o):
    out[t,o] = x[t,:] @ base_w[o,:] + base_b[o]
             + sum_e softmax_e(x[t,:] @ router_w[e,:]) * SCALE
               * sum_r (x[t,:] @ A[e,r,:]) * B[e,o,r]

Strategy:
  - 8192 tokens sharded 8 ways (1024 tokens/core); weights replicated.
  - All operands shipped as bf16 (half the DMA bytes of fp32r; same PE
    throughput: 1 col/cycle). Output returned bf16, upcast on host.
    Norm rel-err ~2e-3, well inside the 2e-2 gate.
  - Per 128-col chunk: YT[j, tok] = [A;router].T @ x accumulated over all
    16 K-chunks (wide-N matmuls), interleaved with a 4-token-chunk base
    matmul wave (4+2 PSUM banks).
  - Softmax/gating WITHOUT PE transposes: exp on ACT straight from PSUM
    (no max-sub needed: logits are ~N(0,1)); per-token sums / broadcast
    done with tiny matmuls (ones / expansion-matrix lhsT); division and
    gating on DVE. The gated rank-32 projection lands directly in the
    [33, tok] lhsT layout the close matmuls need.
  - Close matmul (K=33: gated proj + ones row for bias) accumulates into
    the same PSUM group as the 16 base K-chunks.
  - DMA rings split: scalar=weights/tables, gpsimd=x stream, sync=outputs.
  - 4 warmup matmuls on a zero tile ramp the PE DVFS p-state during the
    fixed ~7us NEFF prologue + first x-chunk DMA.
"""

import os

import numpy as np

import concourse.bacc as bacc
import concourse.bass as bass
import concourse.mybir as mybir
from concourse.bass_utils import run_bass_kernel_spmd
from concourse.tile import TileContext

SCALE = 16.0 / 8.0  # alpha / r

N_CORES = 8
TOK = 8192  # 4 * 2048 tokens total
TPC = TOK // N_CORES  # tokens per core = 1024
D = 2048  # in features
O = 2048  # out features
E = 4
R = 8
ER = E * R  # 32
J = ER + E  # 36: rank-proj rows + router rows
DC = D // 128  # 16 contraction chunks
OCW = 512  # out-feature chunk width (one PSUM bank)
OC = O // OCW  # 4
TC = TPC // 128  # 8 token chunks per core
WAVE = 5  # wave-A token chunks (PSUM banks: WAVE + 1 spare + 2 ypsum)
KP = 128  # close-matmul contraction rows (33 used, padded to full array)

F32 = mybir.dt.float32
BF16 = mybir.dt.bfloat16

# Results of the last device run (for test harness inspection).
last_run_info: dict = {}

_cached = None


def _build_program():
    nc = bacc.Bacc()

    xt_d = nc.declare_dram_parameter("xt", [128, DC * TPC], BF16, isOutput=False)
    wt_d = nc.declare_dram_parameter("wt", [OC, 128, DC * OCW], BF16, isOutput=False)
    w1t_d = nc.declare_dram_parameter("w1t", [128, DC * J], BF16, isOutput=False)
    bcat_d = nc.declare_dram_parameter("bcat", [KP, O], BF16, isOutput=False)
    cst_d = nc.declare_dram_parameter("cst", [E, ER + 4], BF16, isOutput=False)
    out_d = nc.declare_dram_parameter("out", [OC, TC, 128, OCW], BF16, isOutput=True)

    MUL = mybir.AluOpType.mult

    with TileContext(nc) as tc:
        with (
            tc.tile_pool(name="cpool", bufs=1) as cpool,
            tc.tile_pool(name="wpool", bufs=4) as wpool,
            tc.tile_pool(name="opool", bufs=6) as opool,
            tc.tile_pool(name="mpsum", bufs=6, space="PSUM") as mpsum,
            tc.tile_pool(name="ypsum", bufs=2, space="PSUM") as ypsum,
        ):
            # x gets a dedicated ring (gpsimd) so its stream isn't halved by
            # the weight traffic; everything else rides the scalar ring in
            # consumption order (tables, w0 pieces, then bcat/w1..w3 which
            # are not needed before ~60us). Outputs go on sync.
            cstr = cpool.tile([E, ER + 4], BF16)
            nc.scalar.dma_start(out=cstr, in_=cst_d[:, :])
            w1tr = cpool.tile([128, DC * J], BF16)
            nc.scalar.dma_start(out=w1tr, in_=w1t_d[:, :])

            wts = {
                oc: wpool.tile([128, DC * OCW], BF16, name=f"wtr{oc}", tag="wtr")
                for oc in range(OC)
            }
            # w0 in 4 k-block pieces (wave A unlocks per piece).
            for k in range(4):
                nc.scalar.dma_start(
                    out=wts[0][:, k * 4 * OCW : (k + 1) * 4 * OCW],
                    in_=wt_d[0, :, k * 4 * OCW : (k + 1) * 4 * OCW],
                )
            bcatr = cpool.tile([KP, O], BF16)
            nc.scalar.dma_start(out=bcatr, in_=bcat_d[:, :])
            for oc in range(1, OC):
                for h in range(2):
                    nc.scalar.dma_start(
                        out=wts[oc][:, h * 8 * OCW : (h + 1) * 8 * OCW],
                        in_=wt_d[oc, :, h * 8 * OCW : (h + 1) * 8 * OCW],
                    )

            xtr = cpool.tile([128, DC * TPC], BF16)

            def load_x(d0, nd):
                nc.gpsimd.dma_start(
                    out=xtr[:, d0 * TPC : (d0 + nd) * TPC],
                    in_=xt_d[:, d0 * TPC : (d0 + nd) * TPC],
                )

            for dc in range(4):
                load_x(dc, 1)
            for dp in range(2, 8):
                load_x(2 * dp, 2)

            # Warmup: ramp the PE p-state on a zero tile while the first
            # x chunk is still in flight.
            warm_sb = cpool.tile([128, OCW], BF16)
            nc.vector.memset(warm_sb, 0.0)
            # Close lhsT padded to 128 contraction rows: 32 gated-proj rows,
            # the ones row (bias), then zeros (avoids 64-row PE tile mode).
            vwtr = cpool.tile([KP, TPC], BF16)
            for p0 in range(ER, KP, 32):
                nc.vector.memset(vwtr[p0 : p0 + 32, :], 0.0)
            nc.vector.memset(vwtr[ER : ER + 1, :], 1.0)
            warmps = ypsum.tile([128, OCW], F32, name="warmps", tag="yb")
            for _ in range(4):
                nc.tensor.matmul(
                    warmps, lhsT=warm_sb[:, 0:128], rhs=warm_sb, start=True, stop=True
                )

            # --- Router/LoRA-down phase: YT[j, tok] accumulated over all
            # dc, interleaved with wave-A base matmuls in DMA arrival order.
            ytps = [
                ypsum.tile([J, 512], F32, name=f"ytps{th}", tag="yb")
                for th in range(2)
            ]
            psA = {
                t: mpsum.tile([128, OCW], F32, name=f"ps0_{t}", tag="ps")
                for t in range(WAVE)
            }
            for k in range(4):
                for dc in range(4 * k, 4 * k + 4):
                    for th in range(2):
                        nc.tensor.matmul(
                            ytps[th],
                            lhsT=w1tr[:, dc * J : (dc + 1) * J],
                            rhs=xtr[:, dc * TPC + th * 512 : dc * TPC + (th + 1) * 512],
                            start=(dc == 0),
                            stop=(dc == DC - 1),
                        )
                for dc in range(4 * k, 4 * k + 4):
                    for t in range(WAVE):
                        nc.tensor.matmul(
                            psA[t],
                            lhsT=xtr[:, dc * TPC + t * 128 : dc * TPC + (t + 1) * 128],
                            rhs=wts[0][:, dc * OCW : (dc + 1) * OCW],
                            start=(dc == 0),
                            stop=False,
                        )

            # --- Gating chain (no PE transposes).
            # u = exp(logits) straight off PSUM (no max-sub: logits ~N(0,1));
            # sums4 = 0.5-ones matmul replicates s/SCALE onto 4 rows;
            # r4 = SCALE/s via fast approx recip; g = u*r4 on DVE;
            # gb = E8.T@g broadcasts per-expert gates to the 32 rank rows;
            # vw = Y * gb lands directly in the [32, tok] bf16 close layout.
            y_sb = cpool.tile([ER, TPC], BF16)
            u_sb = cpool.tile([E, TPC], BF16)
            r_sb = cpool.tile([E, TPC], F32)
            g_sb = cpool.tile([E, TPC], BF16)
            for th in range(2):
                nc.vector.tensor_copy(
                    y_sb[:, th * 512 : (th + 1) * 512], ytps[th][0:ER, :]
                )
                nc.scalar.activation(
                    u_sb[:, th * 512 : (th + 1) * 512],
                    ytps[th][ER:J, :],
                    mybir.ActivationFunctionType.Exp,
                )

            def open_group(wtr, t, name):
                ps = mpsum.tile([128, OCW], F32, name=name, tag="ps")
                for dc in range(DC):
                    nc.tensor.matmul(
                        ps,
                        lhsT=xtr[:, dc * TPC + t * 128 : dc * TPC + (t + 1) * 128],
                        rhs=wtr[:, dc * OCW : (dc + 1) * OCW],
                        start=(dc == 0),
                        stop=False,
                    )
                return ps

            def open_group_part(ps, wtr, t, dcs):
                for dc in dcs:
                    nc.tensor.matmul(
                        ps,
                        lhsT=xtr[:, dc * TPC + t * 128 : dc * TPC + (t + 1) * 128],
                        rhs=wtr[:, dc * OCW : (dc + 1) * OCW],
                        start=(dc == 0),
                        stop=False,
                    )

            # t5 group split around the chain matmuls to hide DVE latency
            # (psA[0..4] + t5 use all 6 mpsum bufs; ytps rotation hosts the
            # small chain tiles).
            psA[WAVE] = mpsum.tile([128, OCW], F32, name=f"ps0_{WAVE}", tag="ps")
            open_group_part(psA[WAVE], wts[0], WAVE, range(0, 6))

            sums = []
            for th in range(2):
                s = ypsum.tile([E, 512], F32, name=f"sums{th}", tag="yb")
                nc.tensor.matmul(
                    s,
                    lhsT=cstr[0:E, ER : ER + 4],
                    rhs=u_sb[:, th * 512 : (th + 1) * 512],
                    start=True,
                    stop=True,
                )
                sums.append(s)

            open_group_part(psA[WAVE], wts[0], WAVE, range(6, 11))

            for th in range(2):
                nc.vector.reciprocal_approx_fast(
                    out=r_sb[:, th * 512 : (th + 1) * 512], in_=sums[th]
                )
            for th in range(2):
                nc.vector.tensor_tensor(
                    g_sb[:, th * 512 : (th + 1) * 512],
                    u_sb[:, th * 512 : (th + 1) * 512],
                    r_sb[:, th * 512 : (th + 1) * 512],
                    op=MUL,
                )

            open_group_part(psA[WAVE], wts[0], WAVE, range(11, DC))

            gbs = []
            for th in range(2):
                gb = ypsum.tile([ER, 512], F32, name=f"gb{th}", tag="yb")
                nc.tensor.matmul(
                    gb,
                    lhsT=cstr[0:E, 0:ER],
                    rhs=g_sb[:, th * 512 : (th + 1) * 512],
                    start=True,
                    stop=True,
                )
                gbs.append(gb)
            for th in range(2):
                nc.vector.tensor_tensor(
                    vwtr[0:ER, th * 512 : (th + 1) * 512],
                    y_sb[:, th * 512 : (th + 1) * 512],
                    gbs[th],
                    op=MUL,
                )

            # --- Close: fused LoRA-up + bias matmul, cast to bf16, DMA out.
            def close_group(ps, oc, t):
                nc.tensor.matmul(
                    ps,
                    lhsT=vwtr[:, t * 128 : (t + 1) * 128],
                    rhs=bcatr[:, oc * OCW : (oc + 1) * OCW],
                    start=False,
                    stop=True,
                )
                ot = opool.tile([128, OCW], BF16, tag="ot")
                nc.vector.tensor_copy(ot, ps)
                nc.sync.dma_start(out=out_d[oc, t], in_=ot)

            for t in range(WAVE + 1):
                close_group(psA[t], 0, t)
            for t in range(WAVE + 1, TC):
                close_group(open_group(wts[0], t, f"ps0_{t}"), 0, t)
            for oc in range(1, OC):
                for t in range(TC):
                    close_group(open_group(wts[oc], t, f"ps{oc}_{t}"), oc, t)

    nc.compile()
    return nc


def _prep_inputs(x, base_w, base_b, A, B, router_w):
    """Host-side layout prep: per-partition-contiguous bf16 DMA images."""
    import ml_dtypes

    bf16 = ml_dtypes.bfloat16

    x2 = np.ascontiguousarray(x, dtype=np.float32).reshape(TOK, D)
    # xt[core][p, dc*TPC + t] = x2[core*TPC + t, dc*128 + p]
    xv = x2.reshape(N_CORES, TPC, DC, 128)
    xt = (
        np.ascontiguousarray(xv.transpose(0, 3, 2, 1))
        .reshape(N_CORES, 128, DC * TPC)
        .astype(bf16)
    )

    # wt[oc, p, dc*OCW + o] = base_w[oc*OCW + o, dc*128 + p]
    wv = np.ascontiguousarray(base_w, dtype=np.float32).reshape(OC, OCW, DC, 128)
    wt = (
        np.ascontiguousarray(wv.transpose(0, 3, 2, 1))
        .reshape(OC, 128, DC * OCW)
        .astype(bf16)
    )

    # W1 = [A flattened to 32 rows; router_w 4 rows] over D
    W1 = np.concatenate(
        [
            np.asarray(A, dtype=np.float32).reshape(ER, D),
            np.asarray(router_w, np.float32),
        ],
        axis=0,
    )  # [36, D]
    w1t = (
        np.ascontiguousarray(W1.reshape(J, DC, 128).transpose(2, 1, 0))
        .reshape(128, DC * J)
        .astype(bf16)
    )

    # bcat rows 0..31: B[e, o, r] -> [er, o]; row 32: base_b; rows 33..127
    # zero padding (close matmul runs with a full 128-row stationary tile).
    bc = np.zeros((KP, O), np.float32)
    bc[0:ER] = np.asarray(B, dtype=np.float32).transpose(0, 2, 1).reshape(ER, O)
    bc[ER] = np.asarray(base_b, dtype=np.float32)
    bc = bc.astype(bf16)  # [128, O]

    # cst[:, :32] = per-expert expansion (E8); cst[:, 32:36] = 1/SCALE ones
    # block so the sums matmul replicates s/SCALE onto 4 rows and the recip
    # gives SCALE/s directly.
    cst = np.zeros((E, ER + 4), np.float32)
    for e in range(E):
        cst[e, e * R : (e + 1) * R] = 1.0
    cst[:, ER : ER + 4] = 1.0 / SCALE
    cst = cst.astype(bf16)

    return xt, wt, w1t, bc, cst


def kernel(x, base_w, base_b, A, B, router_w):
    global _cached
    if _cached is None:
        _cached = _build_program()
    nc = _cached

    xt, wt, w1t, bc, cst = _prep_inputs(x, base_w, base_b, A, B, router_w)

    in_maps = [
        {"xt": xt[c], "wt": wt, "w1t": w1t, "bcat": bc, "cst": cst}
        for c in range(N_CORES)
    ]
    core_ids = list(range(N_CORES))

    profile = os.environ.get("KERNEL_PROFILE", "0") == "1"
    res = run_bass_kernel_spmd(nc, in_maps, core_ids, trace=profile)

    last_run_info.clear()
    last_run_info["exec_time_ns"] = res.exec_time_ns
    last_run_info["mean_exec_time_ns"] = res.mean_exec_time_ns
    last_run_info["instructions_and_trace"] = res.instructions_and_trace
    last_run_info["profile_json"] = res.profile_json

    # out[core] shape [OC, TC, 128, OCW] bf16 -> tokens x features fp32
    full = np.empty((TOK, O), dtype=np.float32)
    for c in range(N_CORES):
        buf = res.results[c]["out"].astype(np.float32)  # [OC, TC, 128, OCW]
        full[c * TPC : (c + 1) * TPC] = buf.transpose(1, 2, 0, 3).reshape(TPC, O)
    return full.reshape(4, 2048, 2048)


# revision 20
# speedup vs baseline: 1.1397x; 1.0408x over previous
"""MoELoRALinear Trainium2 kernel (8-core data-parallel, Bass/Tile).

Math (per token t, out feature o):
    out[t,o] = x[t,:] @ base_w[o,:] + base_b[o]
             + sum_e softmax_e(x[t,:] @ router_w[e,:]) * SCALE
               * sum_r (x[t,:] @ A[e,r,:]) * B[e,o,r]

Strategy:
  - 8192 tokens sharded 8 ways (1024 tokens/core); weights replicated.
  - All operands shipped as bf16 (half the DMA bytes of fp32r; same PE
    throughput: 1 col/cycle). Output returned bf16, upcast on host.
    Norm rel-err ~2e-3, well inside the 2e-2 gate.
  - Per 128-col chunk: YT[j, tok] = [A;router].T @ x accumulated over all
    16 K-chunks (wide-N matmuls), interleaved with a 4-token-chunk base
    matmul wave (4+2 PSUM banks).
  - Softmax/gating WITHOUT PE transposes: exp on ACT straight from PSUM
    (no max-sub needed: logits are ~N(0,1)); per-token sums / broadcast
    done with tiny matmuls (ones / expansion-matrix lhsT); division and
    gating on DVE. The gated rank-32 projection lands directly in the
    [33, tok] lhsT layout the close matmuls need.
  - Close matmul (K=33: gated proj + ones row for bias) accumulates into
    the same PSUM group as the 16 base K-chunks.
  - DMA rings split: scalar=weights/tables, gpsimd=x stream, sync=outputs.
  - 4 warmup matmuls on a zero tile ramp the PE DVFS p-state during the
    fixed ~7us NEFF prologue + first x-chunk DMA.
"""

import os

import numpy as np

import concourse.bacc as bacc
import concourse.bass as bass
import concourse.mybir as mybir
from concourse.bass_utils import run_bass_kernel_spmd
from concourse.tile import TileContext

SCALE = 16.0 / 8.0  # alpha / r

N_CORES = 8
TOK = 8192  # 4 * 2048 tokens total
TPC = TOK // N_CORES  # tokens per core = 1024
D = 2048  # in features
O = 2048  # out features
E = 4
R = 8
ER = E * R  # 32
J = ER + E  # 36: rank-proj rows + router rows
DC = D // 128  # 16 contraction chunks
OCW = 512  # out-feature chunk width (one PSUM bank)
OC = O // OCW  # 4
TC = TPC // 128  # 8 token chunks per core
WAVE = 4  # wave-A token chunks (PSUM: WAVE + ytps1 + 2 filler groups + 1 ypsum)
KP = 128  # close-matmul contraction rows (33 used, padded to full array)

F32 = mybir.dt.float32
BF16 = mybir.dt.bfloat16

# Results of the last device run (for test harness inspection).
last_run_info: dict = {}

_cached = None


def _build_program():
    nc = bacc.Bacc()

    xt_d = nc.declare_dram_parameter("xt", [128, DC * TPC], BF16, isOutput=False)
    wt_d = nc.declare_dram_parameter("wt", [OC, 128, DC * OCW], BF16, isOutput=False)
    w1t_d = nc.declare_dram_parameter("w1t", [128, DC * J], BF16, isOutput=False)
    bcat_d = nc.declare_dram_parameter("bcat", [KP, O], BF16, isOutput=False)
    cst_d = nc.declare_dram_parameter("cst", [E, ER + 4], BF16, isOutput=False)
    out_d = nc.declare_dram_parameter("out", [OC, TC, 128, OCW], BF16, isOutput=True)

    MUL = mybir.AluOpType.mult

    with TileContext(nc) as tc:
        with (
            tc.tile_pool(name="cpool", bufs=1) as cpool,
            tc.tile_pool(name="wpool", bufs=4) as wpool,
            tc.tile_pool(name="opool", bufs=6) as opool,
            tc.tile_pool(name="mpsum", bufs=7, space="PSUM") as mpsum,
            tc.tile_pool(name="ypsum", bufs=1, space="PSUM") as ypsum,
        ):
            # x gets a dedicated ring (gpsimd) so its stream isn't halved by
            # the weight traffic; everything else rides the scalar ring in
            # consumption order (tables, w0 pieces, then bcat/w1..w3 which
            # are not needed before ~60us). Outputs go on sync.
            cstr = cpool.tile([E, ER + 4], BF16)
            nc.scalar.dma_start(out=cstr, in_=cst_d[:, :])
            w1tr = cpool.tile([128, DC * J], BF16)
            nc.scalar.dma_start(out=w1tr, in_=w1t_d[:, :])

            wts = {
                oc: wpool.tile([128, DC * OCW], BF16, name=f"wtr{oc}", tag="wtr")
                for oc in range(OC)
            }
            # w0 in 4 k-block pieces (wave A unlocks per piece).
            for k in range(4):
                nc.scalar.dma_start(
                    out=wts[0][:, k * 4 * OCW : (k + 1) * 4 * OCW],
                    in_=wt_d[0, :, k * 4 * OCW : (k + 1) * 4 * OCW],
                )
            bcatr = cpool.tile([KP, O], BF16)
            nc.scalar.dma_start(out=bcatr, in_=bcat_d[:, :])
            for oc in range(1, OC):
                for h in range(2):
                    nc.scalar.dma_start(
                        out=wts[oc][:, h * 8 * OCW : (h + 1) * 8 * OCW],
                        in_=wt_d[oc, :, h * 8 * OCW : (h + 1) * 8 * OCW],
                    )

            xtr = cpool.tile([128, DC * TPC], BF16)

            def load_x(d0, nd):
                nc.sync.dma_start(
                    out=xtr[:, d0 * TPC : (d0 + nd) * TPC],
                    in_=xt_d[:, d0 * TPC : (d0 + nd) * TPC],
                )

            for dc in range(4):
                load_x(dc, 1)
            for dp in range(2, 8):
                load_x(2 * dp, 2)

            # Warmup: ramp the PE p-state on a zero tile while the first
            # x chunk is still in flight.
            warm_sb = cpool.tile([128, OCW], BF16)
            nc.vector.memset(warm_sb, 0.0)
            # Close lhsT padded to 128 contraction rows: 32 gated-proj rows,
            # the ones row (bias), then zeros (avoids 64-row PE tile mode).
            vwtr = cpool.tile([KP, TPC], BF16)
            for p0 in range(ER, KP, 32):
                nc.vector.memset(vwtr[p0 : p0 + 32, :], 0.0)
            nc.vector.memset(vwtr[ER : ER + 1, :], 1.0)
            warmps = ypsum.tile([128, OCW], F32, name="warmps", tag="yb")
            for _ in range(8):
                nc.tensor.matmul(
                    warmps, lhsT=warm_sb[:, 0:128], rhs=warm_sb, start=True, stop=True
                )

            # --- Router/LoRA-down phase: YT[j, tok] accumulated over all
            # dc, interleaved with wave-A base matmuls in DMA arrival order.
            # ytps[1] lives in the mpsum pool so ypsum (bufs=1) can rotate
            # the small gating-chain tiles behind it.
            ytps = [
                ypsum.tile([J, 512], F32, name="ytps0", tag="yb"),
                mpsum.tile([J, 512], F32, name="ytps1", tag="ps"),
            ]
            psA = {
                t: mpsum.tile([128, OCW], F32, name=f"ps0_{t}", tag="ps")
                for t in range(WAVE)
            }
            for k in range(4):
                for dc in range(4 * k, 4 * k + 4):
                    for th in range(2):
                        nc.tensor.matmul(
                            ytps[th],
                            lhsT=w1tr[:, dc * J : (dc + 1) * J],
                            rhs=xtr[:, dc * TPC + th * 512 : dc * TPC + (th + 1) * 512],
                            start=(dc == 0),
                            stop=(dc == DC - 1),
                        )
                for dc in range(4 * k, 4 * k + 4):
                    for t in range(WAVE):
                        nc.tensor.matmul(
                            psA[t],
                            lhsT=xtr[:, dc * TPC + t * 128 : dc * TPC + (t + 1) * 128],
                            rhs=wts[0][:, dc * OCW : (dc + 1) * OCW],
                            start=(dc == 0),
                            stop=False,
                        )

            # --- Gating chain (no PE transposes).
            # u = exp(logits) straight off PSUM (no max-sub: logits ~N(0,1));
            # sums4 = 0.5-ones matmul replicates s/SCALE onto 4 rows;
            # r4 = SCALE/s via fast approx recip; g = u*r4 on DVE;
            # gb = E8.T@g broadcasts per-expert gates to the 32 rank rows;
            # vw = Y * gb lands directly in the [32, tok] bf16 close layout.
            y_sb = cpool.tile([ER, TPC], BF16)
            u_sb = cpool.tile([E, TPC], BF16)
            r_sb = cpool.tile([E, TPC], F32)
            g_sb = cpool.tile([E, TPC], BF16)
            for th in range(2):
                nc.vector.tensor_copy(
                    y_sb[:, th * 512 : (th + 1) * 512], ytps[th][0:ER, :]
                )
                nc.scalar.activation(
                    u_sb[:, th * 512 : (th + 1) * 512],
                    ytps[th][ER:J, :],
                    mybir.ActivationFunctionType.Exp,
                )

            def open_group(wtr, t, name):
                ps = mpsum.tile([128, OCW], F32, name=name, tag="ps")
                for dc in range(DC):
                    nc.tensor.matmul(
                        ps,
                        lhsT=xtr[:, dc * TPC + t * 128 : dc * TPC + (t + 1) * 128],
                        rhs=wtr[:, dc * OCW : (dc + 1) * OCW],
                        start=(dc == 0),
                        stop=False,
                    )
                return ps

            def open_group_part(ps, wtr, t, dcs):
                for dc in dcs:
                    nc.tensor.matmul(
                        ps,
                        lhsT=xtr[:, dc * TPC + t * 128 : dc * TPC + (t + 1) * 128],
                        rhs=wtr[:, dc * OCW : (dc + 1) * OCW],
                        start=(dc == 0),
                        stop=False,
                    )

            # The t4/t5/t6 base groups need no new DMA data, so their 48
            # matmuls are interleaved as PE filler around the chain's small
            # matmuls; the single-buf ypsum rotation serializes the chain
            # (sums0 -> recip0 -> sums1 -> recip1 -> gb0 -> vw0 -> gb1 ->
            # vw1), each hop hidden behind ~1.7us of base matmuls.
            def sums_mm(th):
                s = ypsum.tile([E, 512], F32, name=f"sums{th}", tag="yb")
                nc.tensor.matmul(
                    s,
                    lhsT=cstr[0:E, ER : ER + 4],
                    rhs=u_sb[:, th * 512 : (th + 1) * 512],
                    start=True,
                    stop=True,
                )
                return s

            def recip_g(th, s):
                nc.vector.reciprocal_approx_fast(
                    out=r_sb[:, th * 512 : (th + 1) * 512], in_=s
                )
                nc.vector.tensor_tensor(
                    g_sb[:, th * 512 : (th + 1) * 512],
                    u_sb[:, th * 512 : (th + 1) * 512],
                    r_sb[:, th * 512 : (th + 1) * 512],
                    op=MUL,
                )

            def gb_mm(th):
                gb = ypsum.tile([ER, 512], F32, name=f"gb{th}", tag="yb")
                nc.tensor.matmul(
                    gb,
                    lhsT=cstr[0:E, 0:ER],
                    rhs=g_sb[:, th * 512 : (th + 1) * 512],
                    start=True,
                    stop=True,
                )
                return gb

            def vw_tt(th, gb):
                nc.vector.tensor_tensor(
                    vwtr[0:ER, th * 512 : (th + 1) * 512],
                    y_sb[:, th * 512 : (th + 1) * 512],
                    gb,
                    op=MUL,
                )

            psA[WAVE] = open_group(wts[0], WAVE, f"ps0_{WAVE}")
            s0 = sums_mm(0)
            recip_g(0, s0)
            psA[WAVE + 1] = mpsum.tile(
                [128, OCW], F32, name=f"ps0_{WAVE + 1}", tag="ps"
            )
            open_group_part(psA[WAVE + 1], wts[0], WAVE + 1, range(0, 8))
            s1 = sums_mm(1)
            recip_g(1, s1)
            open_group_part(psA[WAVE + 1], wts[0], WAVE + 1, range(8, DC))
            gb0 = gb_mm(0)
            vw_tt(0, gb0)
            psA[WAVE + 2] = mpsum.tile(
                [128, OCW], F32, name=f"ps0_{WAVE + 2}", tag="ps"
            )
            open_group_part(psA[WAVE + 2], wts[0], WAVE + 2, range(0, 8))
            gb1 = gb_mm(1)
            vw_tt(1, gb1)
            open_group_part(psA[WAVE + 2], wts[0], WAVE + 2, range(8, DC))

            # --- Close: fused LoRA-up + bias matmul, cast to bf16, DMA out.
            def close_group(ps, oc, t):
                nc.tensor.matmul(
                    ps,
                    lhsT=vwtr[:, t * 128 : (t + 1) * 128],
                    rhs=bcatr[:, oc * OCW : (oc + 1) * OCW],
                    start=False,
                    stop=True,
                )
                ot = opool.tile([128, OCW], BF16, tag="ot")
                nc.vector.tensor_copy(ot, ps)
                nc.gpsimd.dma_start(out=out_d[oc, t], in_=ot)

            for t in range(WAVE + 3):
                close_group(psA[t], 0, t)
            for t in range(WAVE + 3, TC):
                close_group(open_group(wts[0], t, f"ps0_{t}"), 0, t)
            for oc in range(1, OC):
                for t in range(TC):
                    close_group(open_group(wts[oc], t, f"ps{oc}_{t}"), oc, t)

    nc.compile()
    return nc


def _prep_inputs(x, base_w, base_b, A, B, router_w):
    """Host-side layout prep: per-partition-contiguous bf16 DMA images."""
    import ml_dtypes

    bf16 = ml_dtypes.bfloat16

    x2 = np.ascontiguousarray(x, dtype=np.float32).reshape(TOK, D)
    # xt[core][p, dc*TPC + t] = x2[core*TPC + t, dc*128 + p]
    xv = x2.reshape(N_CORES, TPC, DC, 128)
    xt = (
        np.ascontiguousarray(xv.transpose(0, 3, 2, 1))
        .reshape(N_CORES, 128, DC * TPC)
        .astype(bf16)
    )

    # wt[oc, p, dc*OCW + o] = base_w[oc*OCW + o, dc*128 + p]
    wv = np.ascontiguousarray(base_w, dtype=np.float32).reshape(OC, OCW, DC, 128)
    wt = (
        np.ascontiguousarray(wv.transpose(0, 3, 2, 1))
        .reshape(OC, 128, DC * OCW)
        .astype(bf16)
    )

    # W1 = [A flattened to 32 rows; router_w 4 rows] over D
    W1 = np.concatenate(
        [
            np.asarray(A, dtype=np.float32).reshape(ER, D),
            np.asarray(router_w, np.float32),
        ],
        axis=0,
    )  # [36, D]
    w1t = (
        np.ascontiguousarray(W1.reshape(J, DC, 128).transpose(2, 1, 0))
        .reshape(128, DC * J)
        .astype(bf16)
    )

    # bcat rows 0..31: B[e, o, r] -> [er, o]; row 32: base_b; rows 33..127
    # zero padding (close matmul runs with a full 128-row stationary tile).
    bc = np.zeros((KP, O), np.float32)
    bc[0:ER] = np.asarray(B, dtype=np.float32).transpose(0, 2, 1).reshape(ER, O)
    bc[ER] = np.asarray(base_b, dtype=np.float32)
    bc = bc.astype(bf16)  # [128, O]

    # cst[:, :32] = per-expert expansion (E8); cst[:, 32:36] = 1/SCALE ones
    # block so the sums matmul replicates s/SCALE onto 4 rows and the recip
    # gives SCALE/s directly.
    cst = np.zeros((E, ER + 4), np.float32)
    for e in range(E):
        cst[e, e * R : (e + 1) * R] = 1.0
    cst[:, ER : ER + 4] = 1.0 / SCALE
    cst = cst.astype(bf16)

    return xt, wt, w1t, bc, cst


def kernel(x, base_w, base_b, A, B, router_w):
    global _cached
    if _cached is None:
        _cached = _build_program()
    nc = _cached

    xt, wt, w1t, bc, cst = _prep_inputs(x, base_w, base_b, A, B, router_w)

    in_maps = [
        {"xt": xt[c], "wt": wt, "w1t": w1t, "bcat": bc, "cst": cst}
        for c in range(N_CORES)
    ]
    core_ids = list(range(N_CORES))

    profile = os.environ.get("KERNEL_PROFILE", "0") == "1"
    res = run_bass_kernel_spmd(nc, in_maps, core_ids, trace=profile)

    last_run_info.clear()
    last_run_info["exec_time_ns"] = res.exec_time_ns
    last_run_info["mean_exec_time_ns"] = res.mean_exec_time_ns
    last_run_info["instructions_and_trace"] = res.instructions_and_trace
    last_run_info["profile_json"] = res.profile_json

    # out[core] shape [OC, TC, 128, OCW] bf16 -> tokens x features fp32
    full = np.empty((TOK, O), dtype=np.float32)
    for c in range(N_CORES):
        buf = res.results[c]["out"].astype(np.float32)  # [OC, TC, 128, OCW]
        full[c * TPC : (c + 1) * TPC] = buf.transpose(1, 2, 0, 3).reshape(TPC, O)
    return full.reshape(4, 2048, 2048)
